# revision 2
# baseline (speedup 1.0000x reference)
"""Multi-head attention (B=2, S=2048, D=1024, H=16, RoPE, causal) on 8 trn2 cores.

Sharding: core = b*4 + g  ->  batch b in {0,1}, head-group g (4 heads of 64 dims).
Each core computes q/k/v projections for its 4 heads, RoPE, causal attention,
and a partial output projection (its 256 rows of wo). Host sums the 4 partials
per batch and adds the analytic bias correction bv@wo + bo (softmax rows sum
to 1, so bv contributes a constant vector; bo is a constant vector).

Device layouts are feature-on-partition ("transposed"):
  xt [128, 8, 2048]   xt[p, ko, s] = x[b, s, ko*128 + p]            (bf16)
  qT/kT computed directly as [d', s]; RoPE pair-swap becomes a 32-block
  partition swap because wq/wk columns are host-permuted to [evens|odds]
  per head (valid: scores are invariant under a shared permutation of q,k).
  The swap itself is a PE matmul with a 128x128 XOR-32 permutation matrix.
  scoresT[j, i] = kT.T @ qT per head, two heads packed as concurrent K=64
  row-group matmuls. Softmax skips max-subtraction (|score| <~ 8 here).
  exp on ACT (scale 1/sqrt(64) pre-folded into the q cos/sin tables).
  Denominator comes free from a ones-column appended to v in the AV matmul
  (out rows 0..63 = v.T @ attnT, row 64 = column sums).
  Causality: blocks above the diagonal are skipped, diagonal blocks compute
  only columns >= 128*r and mask a single 128-wide strip.
  y[s, e] = outT.T @ wo accumulates over the 2 c-chunks, fp32 out.
"""

import os

import numpy as np
import ml_dtypes

import concourse.bass as bass
import concourse.bacc as bacc
import concourse.tile as tile
from concourse import mybir

B = 2
S = 2048
D = 1024
H = 16
HD = 64
NCORES = 8
HEADS_PER_CORE = 4
DP = 256  # head dims per core
SEG = 512  # i-seg / s-seg size
NSEG = S // SEG  # 4
NST = S // 128  # 16 s-tiles / j-tiles
KO = D // 128  # 8 contraction tiles

F32 = mybir.dt.float32
BF16 = mybir.dt.bfloat16

_PROGRAM = None


def _build_program():
    nc = bacc.Bacc("TRN2", target_bir_lowering=False, debug=False)

    xt_d = nc.dram_tensor("xt", [128, KO, S], BF16, kind="ExternalInput")
    wq_d = nc.dram_tensor("wqt", [128, KO, DP], BF16, kind="ExternalInput")
    wk_d = nc.dram_tensor("wkt", [128, KO, DP], BF16, kind="ExternalInput")
    wv_d = nc.dram_tensor("wvt", [128, KO, DP], BF16, kind="ExternalInput")
    wo_d = nc.dram_tensor("wot", [128, 2, D], BF16, kind="ExternalInput")
    bq_d = nc.dram_tensor("bqt", [128, 2], F32, kind="ExternalInput")
    bk_d = nc.dram_tensor("bkt", [128, 2], F32, kind="ExternalInput")
    cq_d = nc.dram_tensor("cq", [128, S], BF16, kind="ExternalInput")
    sq_d = nc.dram_tensor("sq", [128, S], BF16, kind="ExternalInput")
    ck_d = nc.dram_tensor("ck", [128, S], BF16, kind="ExternalInput")
    sk_d = nc.dram_tensor("sk", [128, S], BF16, kind="ExternalInput")
    pm_d = nc.dram_tensor("pswap", [128, 128], BF16, kind="ExternalInput")
    cm_d = nc.dram_tensor("cmask", [128, 128], BF16, kind="ExternalInput")
    y_d = nc.dram_tensor("y", [S, D], F32, kind="ExternalOutput")

    with tile.TileContext(nc) as tc:
        with (
            tc.tile_pool(name="const", bufs=1) as const,
            tc.tile_pool(name="persist", bufs=1) as persist,
            tc.tile_pool(name="work", bufs=3) as work,
            tc.tile_pool(name="psmm", bufs=4, space="PSUM") as psmm,
            tc.tile_pool(name="psacc", bufs=4, space="PSUM") as psacc,
        ):
            # ---- constants ----
            xt = []
            for t in range(NSEG):
                xt_t = const.tile([128, KO, SEG], BF16, tag=f"xt{t}")
                nc.sync.dma_start(xt_t[:], xt_d[:, :, t * SEG:(t + 1) * SEG])
                xt.append(xt_t)
            wq = const.tile([128, KO, DP], BF16, tag="wq")
            nc.sync.dma_start(wq[:], wq_d[:])
            wk = const.tile([128, KO, DP], BF16, tag="wk")
            nc.sync.dma_start(wk[:], wk_d[:])
            wv = const.tile([128, KO, DP], BF16, tag="wv")
            nc.sync.dma_start(wv[:], wv_d[:])
            wo = const.tile([128, 2, D], BF16, tag="wo")
            nc.sync.dma_start(wo[:], wo_d[:])
            bq = const.tile([128, 2], F32, tag="bq")
            nc.sync.dma_start(bq[:], bq_d[:])
            bk = const.tile([128, 2], F32, tag="bk")
            nc.sync.dma_start(bk[:], bk_d[:])
            tabs = {}
            for nm, dd in (("cq", cq_d), ("sq", sq_d), ("ck", ck_d), ("sk", sk_d)):
                tt = const.tile([128, S], BF16, tag=nm)
                nc.sync.dma_start(tt[:], dd[:])
                tabs[nm] = tt
            pm = const.tile([128, 128], BF16, tag="pm")
            nc.sync.dma_start(pm[:], pm_d[:])
            cm = const.tile([128, 128], BF16, tag="cm")
            nc.sync.dma_start(cm[:], cm_d[:])

            # ---- phase A: q/k projections + rope (per chunk c, seg t) ----
            qrot = {}
            krot = {}
            for c in range(2):
                for t in range(NSEG):
                    for which, w_sb, b_sb, ctab, stab, store in (
                        ("q", wq, bq, tabs["cq"], tabs["sq"], qrot),
                        ("k", wk, bk, tabs["ck"], tabs["sk"], krot),
                    ):
                        pp = psmm.tile([128, SEG], F32, tag="mm")
                        for ko in range(KO):
                            nc.tensor.matmul(
                                pp[:],
                                w_sb[:, ko, c * 128:(c + 1) * 128],
                                xt[t][:, ko, :],
                                start=(ko == 0),
                                stop=(ko == KO - 1),
                            )
                        qsb = work.tile([128, SEG], BF16, tag="qsb")
                        # add per-partition bias while casting to bf16
                        nc.vector.tensor_scalar_add(qsb[:], pp[:], b_sb[:, c:c + 1])
                        psw = psmm.tile([128, SEG], F32, tag="mm")
                        nc.tensor.matmul(psw[:], pm[:], qsb[:], start=True, stop=True)
                        t1 = work.tile([128, SEG], BF16, tag="t1")
                        nc.vector.tensor_tensor(
                            t1[:], qsb[:], ctab[:, t * SEG:(t + 1) * SEG],
                            mybir.AluOpType.mult)
                        t2 = work.tile([128, SEG], BF16, tag="t2")
                        nc.vector.tensor_tensor(
                            t2[:], psw[:], stab[:, t * SEG:(t + 1) * SEG],
                            mybir.AluOpType.mult)
                        rot = persist.tile([128, SEG], BF16, tag=f"{which}rot_{c}_{t}")
                        nc.vector.tensor_tensor(
                            rot[:], t1[:], t2[:], mybir.AluOpType.add)
                        store[(c, t)] = rot

            # ---- phase A2: v projection (natural layout + ones column) ----
            vt = []
            for st in range(NST):
                pv = psmm.tile([128, SEG], F32, tag="mm")
                for ko in range(KO):
                    nc.tensor.matmul(
                        pv[:, :DP],
                        xt[st // NSEG][:, ko, (st % NSEG) * 128:(st % NSEG) * 128 + 128],
                        wv[:, ko, :],
                        start=(ko == 0),
                        stop=(ko == KO - 1),
                    )
                v_t = persist.tile([128, HEADS_PER_CORE, 66], BF16, tag=f"v_{st}")
                nc.vector.memset(v_t[:, :, 64:66], 1.0)
                nc.vector.tensor_copy(
                    v_t[:, :, 0:64],
                    pv[:, :DP].rearrange("p (h d) -> p h d", h=HEADS_PER_CORE))
                vt.append(v_t)

            # ---- phase B: attention (chunk c = head pair (2c, 2c+1)) ----
            outt = {}
            for c in range(2):
                for t in range(NSEG):
                    pav = [psacc.tile([128, SEG], F32, tag="av", name=f"av_{c}_{t}_{par}")
                           for par in range(2)]
                    njt = 4 * t + 4
                    for jj in range(njt):
                        r = jj - 4 * t  # >= 0 on diagonal blocks
                        col0 = max(0, r) * 128  # first useful i-column
                        w = SEG - col0
                        att = []
                        for par in range(2):
                            ps = psmm.tile([128, SEG], F32, tag="mm")
                            lo, hi = par * 64, par * 64 + 64
                            nc.tensor.matmul(
                                ps[:, col0:],
                                krot[(c, jj // 4)][lo:hi,
                                                   (jj % 4) * 128:(jj % 4) * 128 + 128],
                                qrot[(c, t)][lo:hi, col0:],
                                start=True, stop=True)
                            a = work.tile([128, SEG], BF16, tag=f"attn{par}")
                            nc.scalar.activation(
                                a[:, col0:], ps[:, col0:],
                                mybir.ActivationFunctionType.Exp)
                            if r >= 0:
                                # mask the 128-wide diagonal strip
                                nc.vector.tensor_tensor(
                                    a[:, col0:col0 + 128], a[:, col0:col0 + 128],
                                    cm[:], mybir.AluOpType.mult)
                            att.append(a)
                        for par in range(2):
                            nc.tensor.matmul(
                                pav[par][0:65, col0:],
                                vt[jj][:, 2 * c + par, 0:65],
                                att[par][:, col0:],
                                start=(jj == 0), stop=(jj == njt - 1))
                    ot = persist.tile([128, SEG], BF16, tag=f"outt_{c}_{t}")
                    outt[(c, t)] = ot
                    for par in range(2):
                        rec = work.tile([1, SEG], F32, tag="rec")
                        nc.vector.reciprocal(rec[:], pav[par][64:65, :])
                        bc = work.tile([64, SEG], F32, tag="bc")
                        nc.gpsimd.partition_broadcast(bc[:], rec[:])
                        nc.vector.tensor_tensor(
                            ot[par * 64:par * 64 + 64, :],
                            pav[par][0:64, :], bc[:], mybir.AluOpType.mult)

            # ---- phase C: output projection ----
            for st in range(NST):
                for es in range(2):
                    py = psmm.tile([128, SEG], F32, tag="mm")
                    for co in range(2):
                        nc.tensor.matmul(
                            py[:],
                            outt[(co, st // NSEG)][:, (st % NSEG) * 128:(st % NSEG) * 128 + 128],
                            wo[:, co, es * SEG:(es + 1) * SEG],
                            start=(co == 0), stop=(co == 1))
                    ysb = work.tile([128, SEG], F32, tag="ysb")
                    nc.vector.tensor_copy(ysb[:], py[:])
                    nc.sync.dma_start(
                        y_d[st * 128:(st + 1) * 128, es * SEG:(es + 1) * SEG], ysb[:])

    nc.compile()
    return nc


def _get_program():
    global _PROGRAM
    if _PROGRAM is None:
        _PROGRAM = _build_program()
    return _PROGRAM


def _host_prep(x, wq, bq, wk, bk, wv, bv, wo, bo):
    """Build the 8 per-core input maps (all host-side numpy, cheap)."""
    bf = ml_dtypes.bfloat16
    x = np.asarray(x, np.float32)
    wq = np.asarray(wq, np.float32)
    wk = np.asarray(wk, np.float32)
    wv = np.asarray(wv, np.float32)
    wo = np.asarray(wo, np.float32)
    bq = np.asarray(bq, np.float32)
    bk = np.asarray(bk, np.float32)

    # rope tables, permuted-layout: partition p -> pair index m = p % 32,
    # first half of each 64-block (p%64<32) holds "evens", second "odds".
    m = np.arange(32, dtype=np.float64)
    inv_freq = 1.0 / (10000.0 ** (2.0 * m / HD))  # [32]
    pos = np.arange(S, dtype=np.float64)
    ang = pos[None, :] * inv_freq[:, None]  # [32, S]
    cos32 = np.cos(ang)
    sin32 = np.sin(ang)
    p = np.arange(128)
    cfull = cos32[p % 32, :]  # [128, S]
    sgn = np.where((p % 64) < 32, -1.0, 1.0)[:, None]
    sfull = sin32[p % 32, :] * sgn
    scale = 1.0 / np.sqrt(HD)
    cq_t = (cfull * scale).astype(bf)
    sq_t = (sfull * scale).astype(bf)
    ck_t = cfull.astype(bf)
    sk_t = sfull.astype(bf)

    pswap = np.zeros((128, 128), np.float32)
    pswap[np.arange(128), np.arange(128) ^ 32] = 1.0
    pswap = pswap.astype(bf)

    cmask = (p[:, None] <= np.arange(128)[None, :]).astype(bf)  # keep p <= i'

    in_maps = []
    for core in range(NCORES):
        b, g = divmod(core, HEADS_PER_CORE)
        # permuted columns for q/k: per head [evens, odds]
        colmap = np.concatenate([
            (4 * g + hl) * HD + np.concatenate([np.arange(0, HD, 2),
                                                np.arange(1, HD, 2)])
            for hl in range(HEADS_PER_CORE)
        ])  # [256] global col indices
        vcols = np.arange(g * DP, (g + 1) * DP)

        xt = np.ascontiguousarray(
            x[b].T.reshape(KO, 128, S).transpose(1, 0, 2)).astype(bf)
        wq_t = np.ascontiguousarray(
            wq[:, colmap].reshape(KO, 128, DP).transpose(1, 0, 2)).astype(bf)
        wk_t = np.ascontiguousarray(
            wk[:, colmap].reshape(KO, 128, DP).transpose(1, 0, 2)).astype(bf)
        wv_t = np.ascontiguousarray(
            wv[:, vcols].reshape(KO, 128, DP).transpose(1, 0, 2)).astype(bf)
        wo_t = np.ascontiguousarray(
            wo[vcols, :].reshape(2, 128, D).transpose(1, 0, 2)).astype(bf)
        bq_t = np.ascontiguousarray(bq[colmap].reshape(2, 128).T).astype(np.float32)
        bk_t = np.ascontiguousarray(bk[colmap].reshape(2, 128).T).astype(np.float32)

        in_maps.append({
            "xt": xt, "wqt": wq_t, "wkt": wk_t, "wvt": wv_t, "wot": wo_t,
            "bqt": bq_t, "bkt": bk_t,
            "cq": cq_t, "sq": sq_t, "ck": ck_t, "sk": sk_t,
            "pswap": pswap, "cmask": cmask,
        })
    return in_maps


def _run(nc, in_maps):
    if os.environ.get("BASS_SIM"):
        from concourse.bass_interp import CoreSim
        outs = []
        ncores = int(os.environ.get("BASS_SIM_CORES", "8"))
        for i, m in enumerate(in_maps[:ncores]):
            sim = CoreSim(nc, require_finite=False, require_nnan=False)
            for k, v in m.items():
                sim.tensor(k)[:] = v
            sim.simulate(check_with_hw=False)
            outs.append({"y": np.array(sim.tensor("y"))})
        while len(outs) < len(in_maps):
            outs.append({"y": np.zeros((S, D), np.float32)})
        return outs
    from concourse.bass_utils import run_bass_kernel_spmd
    res = run_bass_kernel_spmd(nc, in_maps, list(range(NCORES)))
    return res.results


def kernel(x, wq, bq, wk, bk, wv, bv, wo, bo):
    nc = _get_program()
    in_maps = _host_prep(x, wq, bq, wk, bk, wv, bv, wo, bo)
    results = _run(nc, in_maps)
    bv = np.asarray(bv, np.float32)
    bo = np.asarray(bo, np.float32)
    wo_f = np.asarray(wo, np.float32)
    corr = bv @ wo_f + bo  # [D]
    y = np.zeros((B, S, D), np.float32)
    for core in range(NCORES):
        b = core // HEADS_PER_CORE
        y[b] += results[core]["y"]
    y += corr[None, None, :]
    return y


# revision 6
# speedup vs baseline: 1.0212x; 1.0212x over previous
"""Multi-head attention (B=2, S=2048, D=1024, H=16, RoPE, causal) on 8 trn2 cores.

Sharding: core = b*4 + g  ->  batch b in {0,1}, head-group g (4 heads of 64 dims).
Each core computes q/k/v projections for its 4 heads, RoPE, causal attention,
and a partial output projection (its 256 rows of wo). Host sums the 4 partials
per batch and adds the analytic bias correction bv@wo + bo (softmax rows sum
to 1, so bv contributes a constant vector; bo is a constant vector).

Device layouts are feature-on-partition ("transposed"):
  xt [128, 8, 2048]   xt[p, ko, s] = x[b, s, ko*128 + p]            (bf16)
  qT/kT computed directly as [d', s]; RoPE pair-swap becomes a 32-block
  partition swap because wq/wk columns are host-permuted to [evens|odds]
  per head (valid: scores are invariant under a shared permutation of q,k).
  The swap itself is a PE matmul with a 128x128 XOR-32 permutation matrix.
  scoresT[j, i] = kT.T @ qT per head; two heads (one 128-part chunk) run as
  concurrent K=64 row-group matmuls into the two banks of one [128,1024]
  PSUM tile, so exp / mask / normalize handle both heads per op.
  Softmax skips max-subtraction (|score| <~ 8 here); exp on ACT with the
  1/sqrt(64) scale pre-folded into the q cos/sin tables. The denominator
  comes free from a ones-column appended to v in the AV matmul (out rows
  0..63 = v.T @ attnT, row 64 = column sums). Causality: blocks above the
  diagonal are skipped, diagonal blocks compute only columns >= 128*r and
  mask a single 128-wide strip.
  y[s, e] = outT.T @ wo accumulated over the 2 c-chunks, DMA'd PSUM->DRAM.
"""

import os

import numpy as np
import ml_dtypes

import concourse.bass as bass
import concourse.bacc as bacc
import concourse.tile as tile
from concourse import mybir

B = 2
S = 2048
D = 1024
H = 16
HD = 64
NCORES = 8
HEADS_PER_CORE = 4
DP = 256  # head dims per core
SEG = 512  # i-seg / s-seg size
NSEG = S // SEG  # 4
NST = S // 128  # 16 s-tiles / j-tiles
KO = D // 128  # 8 contraction tiles

F32 = mybir.dt.float32
BF16 = mybir.dt.bfloat16

_PROGRAMS = {}


def _build_program(with_qk_bias):
    nc = bacc.Bacc("TRN2", target_bir_lowering=False, debug=False)

    xt_d = nc.dram_tensor("xt", [128, KO, S], BF16, kind="ExternalInput")
    wq_d = nc.dram_tensor("wqt", [128, KO, DP], BF16, kind="ExternalInput")
    wk_d = nc.dram_tensor("wkt", [128, KO, DP], BF16, kind="ExternalInput")
    wv_d = nc.dram_tensor("wvt", [128, KO, DP], BF16, kind="ExternalInput")
    wo_d = nc.dram_tensor("wot", [128, 2, D], BF16, kind="ExternalInput")
    bq_d = nc.dram_tensor("bqt", [128, 2], F32, kind="ExternalInput")
    bk_d = nc.dram_tensor("bkt", [128, 2], F32, kind="ExternalInput")
    cq_d = nc.dram_tensor("cq", [128, S], BF16, kind="ExternalInput")
    sq_d = nc.dram_tensor("sq", [128, S], BF16, kind="ExternalInput")
    ck_d = nc.dram_tensor("ck", [128, S], BF16, kind="ExternalInput")
    sk_d = nc.dram_tensor("sk", [128, S], BF16, kind="ExternalInput")
    pm_d = nc.dram_tensor("pswap", [128, 128], BF16, kind="ExternalInput")
    cm_d = nc.dram_tensor("cmask", [128, 128], BF16, kind="ExternalInput")
    y_d = nc.dram_tensor("y", [S, D], F32, kind="ExternalOutput")

    with tile.TileContext(nc) as tc:
        with (
            tc.tile_pool(name="const", bufs=1) as const,
            tc.tile_pool(name="persist", bufs=1) as persist,
            tc.tile_pool(name="work", bufs=4) as work,
            tc.tile_pool(name="psmm", bufs=2, space="PSUM") as psmm,
            tc.tile_pool(name="pssc", bufs=2, space="PSUM") as pssc,
            tc.tile_pool(name="psacc", bufs=2, space="PSUM") as psacc,
        ):
            # ---- constants ----
            xt = []
            for t in range(NSEG):
                xt_t = const.tile([128, KO, SEG], BF16, tag=f"xt{t}")
                nc.sync.dma_start(xt_t[:], xt_d[:, :, t * SEG:(t + 1) * SEG])
                xt.append(xt_t)
            wq = const.tile([128, KO, DP], BF16, tag="wq")
            nc.sync.dma_start(wq[:], wq_d[:])
            wk = const.tile([128, KO, DP], BF16, tag="wk")
            nc.sync.dma_start(wk[:], wk_d[:])
            wv = const.tile([128, KO, DP], BF16, tag="wv")
            nc.sync.dma_start(wv[:], wv_d[:])
            wo = const.tile([128, 2, D], BF16, tag="wo")
            nc.sync.dma_start(wo[:], wo_d[:])
            if with_qk_bias:
                bq = const.tile([128, 2], F32, tag="bq")
                nc.sync.dma_start(bq[:], bq_d[:])
                bk = const.tile([128, 2], F32, tag="bk")
                nc.sync.dma_start(bk[:], bk_d[:])
            tabs = {}
            for nm, dd in (("cq", cq_d), ("sq", sq_d), ("ck", ck_d), ("sk", sk_d)):
                tt = const.tile([128, S], BF16, tag=nm)
                nc.sync.dma_start(tt[:], dd[:])
                tabs[nm] = tt
            pm = const.tile([128, 128], BF16, tag="pm")
            nc.sync.dma_start(pm[:], pm_d[:])
            cm = const.tile([128, 128], BF16, tag="cm")
            nc.sync.dma_start(cm[:], cm_d[:])

            # ---- phase A: q/k projections + rope (per chunk c, seg t) ----
            qrot = {}
            krot = {}
            for c in range(2):
                for t in range(NSEG):
                    for which, w_sb, bofs, ctab, stab, store in (
                        ("q", wq, 0, tabs["cq"], tabs["sq"], qrot),
                        ("k", wk, 1, tabs["ck"], tabs["sk"], krot),
                    ):
                        pp = psmm.tile([128, SEG], F32, tag="mm",
                                       name=f"p{which}_{c}_{t}")
                        for ko in range(KO):
                            nc.tensor.matmul(
                                pp[:],
                                w_sb[:, ko, c * 128:(c + 1) * 128],
                                xt[t][:, ko, :],
                                start=(ko == 0),
                                stop=(ko == KO - 1),
                            )
                        qsb = work.tile([128, SEG], BF16, tag="qsb")
                        if with_qk_bias:
                            b_sb = bq if which == "q" else bk
                            nc.vector.tensor_scalar_add(
                                qsb[:], pp[:], b_sb[:, c:c + 1])
                        else:
                            nc.vector.tensor_copy(qsb[:], pp[:])
                        psw = psmm.tile([128, SEG], F32, tag="mm",
                                        name=f"psw{which}_{c}_{t}")
                        nc.tensor.matmul(psw[:], pm[:], qsb[:], start=True, stop=True)
                        t1 = work.tile([128, SEG], BF16, tag="t1")
                        nc.vector.tensor_tensor(
                            t1[:], qsb[:], ctab[:, t * SEG:(t + 1) * SEG],
                            mybir.AluOpType.mult)
                        t2 = work.tile([128, SEG], BF16, tag="t2")
                        nc.vector.tensor_tensor(
                            t2[:], psw[:], stab[:, t * SEG:(t + 1) * SEG],
                            mybir.AluOpType.mult)
                        rot = persist.tile([128, SEG], BF16, tag=f"{which}rot_{c}_{t}")
                        nc.vector.tensor_tensor(
                            rot[:], t1[:], t2[:], mybir.AluOpType.add)
                        store[(c, t)] = rot

            # ---- phase A2: v projection (natural layout + ones column) ----
            vt = []
            for st in range(NST):
                pv = psmm.tile([128, SEG], F32, tag="mm", name=f"pv_{st}")
                for ko in range(KO):
                    nc.tensor.matmul(
                        pv[:, :DP],
                        xt[st // NSEG][:, ko, (st % NSEG) * 128:(st % NSEG) * 128 + 128],
                        wv[:, ko, :],
                        start=(ko == 0),
                        stop=(ko == KO - 1),
                    )
                v_t = persist.tile([128, HEADS_PER_CORE, 66], BF16, tag=f"v_{st}")
                nc.vector.memset(v_t[:, :, 64:66], 1.0)
                nc.vector.tensor_copy(
                    v_t[:, :, 0:64],
                    pv[:, :DP].rearrange("p (h d) -> p h d", h=HEADS_PER_CORE))
                vt.append(v_t)

            # ---- phase B + C interleaved by i-seg t ----
            outt = {}
            for t in range(NSEG):
                for c in range(2):
                    pav = [psacc.tile([128, SEG], F32, tag="av",
                                      name=f"av_{c}_{t}_{par}")
                           for par in range(2)]
                    njt = 4 * t + 4
                    for jj in range(njt):
                        r = jj - 4 * t  # >= 0 on diagonal blocks
                        col0 = max(0, r) * 128  # first useful i-column
                        a = work.tile([128, 2, SEG], BF16, tag="attn")
                        for par in range(2):
                            ps = pssc.tile([128, SEG], F32, tag="sc",
                                           name=f"sc_{c}_{t}_{jj}_{par}")
                            lo, hi = par * 64, par * 64 + 64
                            nc.tensor.matmul(
                                ps[:, col0:],
                                krot[(c, jj // 4)][lo:hi,
                                                   (jj % 4) * 128:(jj % 4) * 128 + 128],
                                qrot[(c, t)][lo:hi, col0:],
                                start=True, stop=True)
                            nc.scalar.activation(
                                a[:, par, col0:], ps[:, col0:],
                                mybir.ActivationFunctionType.Exp)
                        if r >= 0:
                            # mask the 128-wide diagonal strip (both parities)
                            nc.vector.tensor_tensor(
                                a[:, :, col0:col0 + 128], a[:, :, col0:col0 + 128],
                                cm[:, None, :].to_broadcast((128, 2, 128)),
                                mybir.AluOpType.mult)
                        for par in range(2):
                            nc.tensor.matmul(
                                pav[par][0:65, col0:],
                                vt[jj][:, 2 * c + par, 0:65],
                                a[:, par, col0:],
                                start=(jj == 0), stop=(jj == njt - 1))
                    ot = persist.tile([128, SEG], BF16, tag=f"outt_{c}_{t}")
                    outt[(c, t)] = ot
                    for par in range(2):
                        rec = work.tile([1, SEG], F32, tag="rec")
                        nc.vector.reciprocal(rec[:], pav[par][64:65, :])
                        bc = work.tile([64, SEG], F32, tag="bc")
                        nc.gpsimd.partition_broadcast(bc[:], rec[:])
                        nc.vector.tensor_tensor(
                            ot[par * 64:par * 64 + 64, :],
                            pav[par][0:64, :], bc[:], mybir.AluOpType.mult)
                # ---- output projection for the 4 s-tiles of this seg ----
                for sl in range(4):
                    st = 4 * t + sl
                    for es in range(2):
                        py = psmm.tile([128, SEG], F32, tag="mm",
                                       name=f"py_{st}_{es}")
                        for co in range(2):
                            nc.tensor.matmul(
                                py[:],
                                outt[(co, t)][:, sl * 128:sl * 128 + 128],
                                wo[:, co, es * SEG:(es + 1) * SEG],
                                start=(co == 0), stop=(co == 1))
                        ysb = work.tile([128, SEG], F32, tag="ysb")
                        nc.any.tensor_copy(ysb[:], py[:])
                        nc.sync.dma_start(
                            y_d[st * 128:(st + 1) * 128, es * SEG:(es + 1) * SEG],
                            ysb[:])

    nc.compile()
    return nc


def _get_program(with_qk_bias=False):
    if with_qk_bias not in _PROGRAMS:
        _PROGRAMS[with_qk_bias] = _build_program(with_qk_bias)
    return _PROGRAMS[with_qk_bias]


def _host_prep(x, wq, bq, wk, bk, wv, bv, wo, bo):
    """Build the 8 per-core input maps (all host-side numpy, cheap)."""
    bf = ml_dtypes.bfloat16
    x = np.asarray(x, np.float32)
    wq = np.asarray(wq, np.float32)
    wk = np.asarray(wk, np.float32)
    wv = np.asarray(wv, np.float32)
    wo = np.asarray(wo, np.float32)
    bq = np.asarray(bq, np.float32)
    bk = np.asarray(bk, np.float32)

    # rope tables, permuted-layout: partition p -> pair index m = p % 32,
    # first half of each 64-block (p%64<32) holds "evens", second "odds".
    m = np.arange(32, dtype=np.float64)
    inv_freq = 1.0 / (10000.0 ** (2.0 * m / HD))  # [32]
    pos = np.arange(S, dtype=np.float64)
    ang = pos[None, :] * inv_freq[:, None]  # [32, S]
    cos32 = np.cos(ang)
    sin32 = np.sin(ang)
    p = np.arange(128)
    cfull = cos32[p % 32, :]  # [128, S]
    sgn = np.where((p % 64) < 32, -1.0, 1.0)[:, None]
    sfull = sin32[p % 32, :] * sgn
    scale = 1.0 / np.sqrt(HD)
    cq_t = (cfull * scale).astype(bf)
    sq_t = (sfull * scale).astype(bf)
    ck_t = cfull.astype(bf)
    sk_t = sfull.astype(bf)

    pswap = np.zeros((128, 128), np.float32)
    pswap[np.arange(128), np.arange(128) ^ 32] = 1.0
    pswap = pswap.astype(bf)

    cmask = (p[:, None] <= np.arange(128)[None, :]).astype(bf)  # keep p <= i'

    in_maps = []
    for core in range(NCORES):
        b, g = divmod(core, HEADS_PER_CORE)
        # permuted columns for q/k: per head [evens, odds]
        colmap = np.concatenate([
            (4 * g + hl) * HD + np.concatenate([np.arange(0, HD, 2),
                                                np.arange(1, HD, 2)])
            for hl in range(HEADS_PER_CORE)
        ])  # [256] global col indices
        vcols = np.arange(g * DP, (g + 1) * DP)

        xt = np.ascontiguousarray(
            x[b].T.reshape(KO, 128, S).transpose(1, 0, 2)).astype(bf)
        wq_t = np.ascontiguousarray(
            wq[:, colmap].reshape(KO, 128, DP).transpose(1, 0, 2)).astype(bf)
        wk_t = np.ascontiguousarray(
            wk[:, colmap].reshape(KO, 128, DP).transpose(1, 0, 2)).astype(bf)
        wv_t = np.ascontiguousarray(
            wv[:, vcols].reshape(KO, 128, DP).transpose(1, 0, 2)).astype(bf)
        wo_t = np.ascontiguousarray(
            wo[vcols, :].reshape(2, 128, D).transpose(1, 0, 2)).astype(bf)
        bq_t = np.ascontiguousarray(bq[colmap].reshape(2, 128).T).astype(np.float32)
        bk_t = np.ascontiguousarray(bk[colmap].reshape(2, 128).T).astype(np.float32)

        in_maps.append({
            "xt": xt, "wqt": wq_t, "wkt": wk_t, "wvt": wv_t, "wot": wo_t,
            "bqt": bq_t, "bkt": bk_t,
            "cq": cq_t, "sq": sq_t, "ck": ck_t, "sk": sk_t,
            "pswap": pswap, "cmask": cmask,
        })
    return in_maps


def _run(nc, in_maps):
    if os.environ.get("BASS_SIM"):
        from concourse.bass_interp import CoreSim
        outs = []
        ncores = int(os.environ.get("BASS_SIM_CORES", "8"))
        for i, m in enumerate(in_maps[:ncores]):
            sim = CoreSim(nc, require_finite=False, require_nnan=False)
            for k, v in m.items():
                sim.tensor(k)[:] = v
            sim.simulate(check_with_hw=False)
            outs.append({"y": np.array(sim.tensor("y"))})
        while len(outs) < len(in_maps):
            outs.append({"y": np.zeros((S, D), np.float32)})
        return outs
    from concourse.bass_utils import run_bass_kernel_spmd
    res = run_bass_kernel_spmd(nc, in_maps, list(range(NCORES)))
    return res.results


def kernel(x, wq, bq, wk, bk, wv, bv, wo, bo):
    with_qk_bias = bool(np.any(np.asarray(bq)) or np.any(np.asarray(bk)))
    nc = _get_program(with_qk_bias)
    in_maps = _host_prep(x, wq, bq, wk, bk, wv, bv, wo, bo)
    results = _run(nc, in_maps)
    bv = np.asarray(bv, np.float32)
    bo = np.asarray(bo, np.float32)
    wo_f = np.asarray(wo, np.float32)
    corr = bv @ wo_f + bo  # [D]
    y = np.zeros((B, S, D), np.float32)
    for core in range(NCORES):
        b = core // HEADS_PER_CORE
        y[b] += results[core]["y"]
    y += corr[None, None, :]
    return y


# revision 10
# speedup vs baseline: 1.1566x; 1.1326x over previous
"""Multi-head attention (B=2, S=2048, D=1024, H=16, RoPE, causal) on 8 trn2 cores.

Sharding: core = b*4 + g  ->  batch b in {0,1}, head-group g (4 heads of 64 dims).
Each core computes q/k/v projections for its 4 heads, RoPE, causal attention,
and a partial output projection (its 256 rows of wo). Host sums the 4 partials
per batch and adds the analytic bias correction bv@wo + bo (softmax rows sum
to 1, so bv contributes a constant vector; bo is a constant vector).

Device layouts are feature-on-partition ("transposed"):
  xt [128, 8, 2048]   xt[p, ko, s] = x[b, s, ko*128 + p]            (bf16)
  qT/kT computed directly as [d', s]; RoPE pair-swap becomes a 32-block
  partition swap because wq/wk columns are host-permuted to [evens|odds]
  per head (valid: scores are invariant under a shared permutation of q,k).
  The swap itself is a PE matmul with a 128x128 XOR-32 permutation matrix.
  scoresT[j, i] = kT.T @ qT per head; two heads (one 128-part chunk) run as
  concurrent K=64 row-group matmuls into the two banks of one [128,1024]
  PSUM tile, so exp / mask / normalize handle both heads per op.
  Softmax skips max-subtraction (|score| <~ 8 here); exp on ACT with the
  1/sqrt(64) scale pre-folded into the q cos/sin tables. The denominator
  comes free from a ones-column appended to v in the AV matmul (out rows
  0..63 = v.T @ attnT, row 64 = column sums). Causality: blocks above the
  diagonal are skipped, diagonal blocks compute only columns >= 128*r and
  mask a single 128-wide strip.
  y[s, e] = outT.T @ wo accumulated over the 2 c-chunks, DMA'd PSUM->DRAM.
"""

import os

import numpy as np
import ml_dtypes

import concourse.bass as bass
import concourse.bacc as bacc
import concourse.tile as tile
from concourse import mybir

B = 2
S = 2048
D = 1024
H = 16
HD = 64
NCORES = 8
HEADS_PER_CORE = 4
DP = 256  # head dims per core
SEG = 512  # i-seg / s-seg size
NSEG = S // SEG  # 4
NST = S // 128  # 16 s-tiles / j-tiles
KO = D // 128  # 8 contraction tiles

F32 = mybir.dt.float32
BF16 = mybir.dt.bfloat16

_PROGRAMS = {}


def _build_program(with_qk_bias):
    nc = bacc.Bacc("TRN2", target_bir_lowering=False, debug=False)

    xt_d = nc.dram_tensor("xt", [128, KO, S], BF16, kind="ExternalInput")
    wq_d = nc.dram_tensor("wqt", [128, KO, DP], BF16, kind="ExternalInput")
    wk_d = nc.dram_tensor("wkt", [128, KO, DP], BF16, kind="ExternalInput")
    wv_d = nc.dram_tensor("wvt", [128, KO, DP], BF16, kind="ExternalInput")
    wo_d = nc.dram_tensor("wot", [128, 2, D], BF16, kind="ExternalInput")
    bq_d = nc.dram_tensor("bqt", [128, 2], F32, kind="ExternalInput")
    bk_d = nc.dram_tensor("bkt", [128, 2], F32, kind="ExternalInput")
    cq_d = nc.dram_tensor("cq", [128, S], BF16, kind="ExternalInput")
    sq_d = nc.dram_tensor("sq", [128, S], BF16, kind="ExternalInput")
    ck_d = nc.dram_tensor("ck", [128, S], BF16, kind="ExternalInput")
    sk_d = nc.dram_tensor("sk", [128, S], BF16, kind="ExternalInput")
    pm_d = nc.dram_tensor("pswap", [128, 128], BF16, kind="ExternalInput")
    cm_d = nc.dram_tensor("cmask", [128, 128], BF16, kind="ExternalInput")
    y_d = nc.dram_tensor("y", [S, D], F32, kind="ExternalOutput")

    with tile.TileContext(nc) as tc:
        with (
            tc.tile_pool(name="const", bufs=1) as const,
            tc.tile_pool(name="persist", bufs=1) as persist,
            tc.tile_pool(name="work", bufs=4) as work,
            tc.tile_pool(name="psmm", bufs=2, space="PSUM") as psmm,
            tc.tile_pool(name="pssc", bufs=2, space="PSUM") as pssc,
            tc.tile_pool(name="psacc", bufs=2, space="PSUM") as psacc,
        ):
            # ---- constants ----
            xt = []
            for t in range(NSEG):
                xt_t = const.tile([128, KO, SEG], BF16, tag=f"xt{t}")
                nc.sync.dma_start(xt_t[:], xt_d[:, :, t * SEG:(t + 1) * SEG])
                xt.append(xt_t)
            wq = const.tile([128, KO, DP], BF16, tag="wq")
            nc.sync.dma_start(wq[:], wq_d[:])
            wk = const.tile([128, KO, DP], BF16, tag="wk")
            nc.sync.dma_start(wk[:], wk_d[:])
            wv = const.tile([128, KO, DP], BF16, tag="wv")
            nc.sync.dma_start(wv[:], wv_d[:])
            wo = const.tile([128, 2, D], BF16, tag="wo")
            nc.sync.dma_start(wo[:], wo_d[:])
            if with_qk_bias:
                bq = const.tile([128, 2], F32, tag="bq")
                nc.sync.dma_start(bq[:], bq_d[:])
                bk = const.tile([128, 2], F32, tag="bk")
                nc.sync.dma_start(bk[:], bk_d[:])
            tabs = {}
            for nm, dd in (("cq", cq_d), ("sq", sq_d), ("ck", ck_d), ("sk", sk_d)):
                tt = const.tile([128, S], BF16, tag=nm)
                nc.sync.dma_start(tt[:], dd[:])
                tabs[nm] = tt
            pm = const.tile([128, 128], BF16, tag="pm")
            nc.sync.dma_start(pm[:], pm_d[:])
            cm = const.tile([128, 128], BF16, tag="cm")
            nc.sync.dma_start(cm[:], cm_d[:])

            # ---- phase A: q/k projections + rope (per chunk c, seg t) ----
            qrot = {}
            krot = {}
            for c in range(2):
                for t in range(NSEG):
                    for which, w_sb, bofs, ctab, stab, store in (
                        ("q", wq, 0, tabs["cq"], tabs["sq"], qrot),
                        ("k", wk, 1, tabs["ck"], tabs["sk"], krot),
                    ):
                        pp = psmm.tile([128, SEG], F32, tag="mm",
                                       name=f"p{which}_{c}_{t}")
                        for ko in range(KO):
                            nc.tensor.matmul(
                                pp[:],
                                w_sb[:, ko, c * 128:(c + 1) * 128],
                                xt[t][:, ko, :],
                                start=(ko == 0),
                                stop=(ko == KO - 1),
                            )
                        qsb = work.tile([128, SEG], BF16, tag="qsb")
                        if with_qk_bias:
                            b_sb = bq if which == "q" else bk
                            nc.vector.tensor_scalar_add(
                                qsb[:], pp[:], b_sb[:, c:c + 1])
                        else:
                            nc.vector.tensor_copy(qsb[:], pp[:])
                        psw = psmm.tile([128, SEG], F32, tag="mm",
                                        name=f"psw{which}_{c}_{t}")
                        nc.tensor.matmul(psw[:], pm[:], qsb[:], start=True, stop=True)
                        t1 = work.tile([128, SEG], BF16, tag="t1")
                        nc.vector.tensor_tensor(
                            t1[:], qsb[:], ctab[:, t * SEG:(t + 1) * SEG],
                            mybir.AluOpType.mult)
                        t2 = work.tile([128, SEG], BF16, tag="t2")
                        nc.vector.tensor_tensor(
                            t2[:], psw[:], stab[:, t * SEG:(t + 1) * SEG],
                            mybir.AluOpType.mult)
                        rot = persist.tile([128, SEG], BF16, tag=f"{which}rot_{c}_{t}")
                        nc.vector.tensor_tensor(
                            rot[:], t1[:], t2[:], mybir.AluOpType.add)
                        store[(c, t)] = rot

            # ---- phase A2: v projection (natural layout + ones column) ----
            vt = []
            for st in range(NST):
                pv = psmm.tile([128, SEG], F32, tag="mm", name=f"pv_{st}")
                for ko in range(KO):
                    nc.tensor.matmul(
                        pv[:, :DP],
                        xt[st // NSEG][:, ko, (st % NSEG) * 128:(st % NSEG) * 128 + 128],
                        wv[:, ko, :],
                        start=(ko == 0),
                        stop=(ko == KO - 1),
                    )
                v_t = persist.tile([128, HEADS_PER_CORE, 66], BF16, tag=f"v_{st}")
                nc.vector.memset(v_t[:, :, 64:66], 1.0)
                nc.vector.tensor_copy(
                    v_t[:, :, 0:64],
                    pv[:, :DP].rearrange("p (h d) -> p h d", h=HEADS_PER_CORE))
                vt.append(v_t)

            # ---- phase B + C interleaved by i-seg t ----
            outt = {}
            for t in range(NSEG):
                for c in range(2):
                    pav = [psacc.tile([128, SEG], F32, tag="av",
                                      name=f"av_{c}_{t}_{par}")
                           for par in range(2)]
                    njt = 4 * t + 4
                    for jj in range(njt):
                        r = jj - 4 * t  # >= 0 on diagonal blocks
                        col0 = max(0, r) * 128  # first useful i-column
                        a = work.tile([128, 2, SEG], BF16, tag="attn")
                        ps = pssc.tile([128, 2, SEG], F32, tag="sc",
                                       name=f"sc_{c}_{t}_{jj}")
                        for par in range(2):
                            lo, hi = par * 64, par * 64 + 64
                            nc.tensor.matmul(
                                ps[:, par, col0:],
                                krot[(c, jj // 4)][lo:hi,
                                                   (jj % 4) * 128:(jj % 4) * 128 + 128],
                                qrot[(c, t)][lo:hi, col0:],
                                start=True, stop=True)
                        nc.scalar.activation(
                            a[:, :, col0:], ps[:, :, col0:],
                            mybir.ActivationFunctionType.Exp)
                        if r >= 0:
                            # mask the 128-wide diagonal strip (both parities)
                            nc.vector.tensor_tensor(
                                a[:, :, col0:col0 + 128], a[:, :, col0:col0 + 128],
                                cm[:, None, :].to_broadcast((128, 2, 128)),
                                mybir.AluOpType.mult)
                        for par in range(2):
                            nc.tensor.matmul(
                                pav[par][0:65, col0:],
                                vt[jj][:, 2 * c + par, 0:65],
                                a[:, par, col0:],
                                start=(jj == 0), stop=(jj == njt - 1))
                    ot = persist.tile([128, SEG], BF16, tag=f"outt_{c}_{t}")
                    outt[(c, t)] = ot
                    for par in range(2):
                        # 1/den = exp(-ln(den)) on ACT (reciprocal is an
                        # 8-cycle iterative op on DVE; exp+ln share one table)
                        lg = work.tile([1, SEG], F32, tag="lg")
                        nc.scalar.activation(
                            lg[:], pav[par][64:65, :],
                            mybir.ActivationFunctionType.Ln)
                        rec = work.tile([1, SEG], F32, tag="rec")
                        nc.scalar.activation(
                            rec[:], lg[:],
                            mybir.ActivationFunctionType.Exp, scale=-1.0)
                        bc = work.tile([64, SEG], F32, tag="bc")
                        nc.gpsimd.partition_broadcast(bc[:], rec[:])
                        nc.vector.tensor_tensor(
                            ot[par * 64:par * 64 + 64, :],
                            pav[par][0:64, :], bc[:], mybir.AluOpType.mult)
                # ---- output projection for the 4 s-tiles of this seg ----
                for sl in range(4):
                    st = 4 * t + sl
                    for es in range(2):
                        py = psmm.tile([128, SEG], F32, tag="mm",
                                       name=f"py_{st}_{es}")
                        for co in range(2):
                            nc.tensor.matmul(
                                py[:],
                                outt[(co, t)][:, sl * 128:sl * 128 + 128],
                                wo[:, co, es * SEG:(es + 1) * SEG],
                                start=(co == 0), stop=(co == 1))
                        ysb = work.tile([128, SEG], F32, tag="ysb")
                        nc.any.tensor_copy(ysb[:], py[:])
                        nc.sync.dma_start(
                            y_d[st * 128:(st + 1) * 128, es * SEG:(es + 1) * SEG],
                            ysb[:])

    nc.compile()
    return nc


def _get_program(with_qk_bias=False):
    if with_qk_bias not in _PROGRAMS:
        _PROGRAMS[with_qk_bias] = _build_program(with_qk_bias)
    return _PROGRAMS[with_qk_bias]


def _host_prep(x, wq, bq, wk, bk, wv, bv, wo, bo):
    """Build the 8 per-core input maps (all host-side numpy, cheap)."""
    bf = ml_dtypes.bfloat16
    x = np.asarray(x, np.float32)
    wq = np.asarray(wq, np.float32)
    wk = np.asarray(wk, np.float32)
    wv = np.asarray(wv, np.float32)
    wo = np.asarray(wo, np.float32)
    bq = np.asarray(bq, np.float32)
    bk = np.asarray(bk, np.float32)

    # rope tables, permuted-layout: partition p -> pair index m = p % 32,
    # first half of each 64-block (p%64<32) holds "evens", second "odds".
    m = np.arange(32, dtype=np.float64)
    inv_freq = 1.0 / (10000.0 ** (2.0 * m / HD))  # [32]
    pos = np.arange(S, dtype=np.float64)
    ang = pos[None, :] * inv_freq[:, None]  # [32, S]
    cos32 = np.cos(ang)
    sin32 = np.sin(ang)
    p = np.arange(128)
    cfull = cos32[p % 32, :]  # [128, S]
    sgn = np.where((p % 64) < 32, -1.0, 1.0)[:, None]
    sfull = sin32[p % 32, :] * sgn
    scale = 1.0 / np.sqrt(HD)
    cq_t = (cfull * scale).astype(bf)
    sq_t = (sfull * scale).astype(bf)
    ck_t = cfull.astype(bf)
    sk_t = sfull.astype(bf)

    pswap = np.zeros((128, 128), np.float32)
    pswap[np.arange(128), np.arange(128) ^ 32] = 1.0
    pswap = pswap.astype(bf)

    cmask = (p[:, None] <= np.arange(128)[None, :]).astype(bf)  # keep p <= i'

    in_maps = []
    for core in range(NCORES):
        b, g = divmod(core, HEADS_PER_CORE)
        # permuted columns for q/k: per head [evens, odds]
        colmap = np.concatenate([
            (4 * g + hl) * HD + np.concatenate([np.arange(0, HD, 2),
                                                np.arange(1, HD, 2)])
            for hl in range(HEADS_PER_CORE)
        ])  # [256] global col indices
        vcols = np.arange(g * DP, (g + 1) * DP)

        xt = np.ascontiguousarray(
            x[b].T.reshape(KO, 128, S).transpose(1, 0, 2)).astype(bf)
        wq_t = np.ascontiguousarray(
            wq[:, colmap].reshape(KO, 128, DP).transpose(1, 0, 2)).astype(bf)
        wk_t = np.ascontiguousarray(
            wk[:, colmap].reshape(KO, 128, DP).transpose(1, 0, 2)).astype(bf)
        wv_t = np.ascontiguousarray(
            wv[:, vcols].reshape(KO, 128, DP).transpose(1, 0, 2)).astype(bf)
        wo_t = np.ascontiguousarray(
            wo[vcols, :].reshape(2, 128, D).transpose(1, 0, 2)).astype(bf)
        bq_t = np.ascontiguousarray(bq[colmap].reshape(2, 128).T).astype(np.float32)
        bk_t = np.ascontiguousarray(bk[colmap].reshape(2, 128).T).astype(np.float32)

        in_maps.append({
            "xt": xt, "wqt": wq_t, "wkt": wk_t, "wvt": wv_t, "wot": wo_t,
            "bqt": bq_t, "bkt": bk_t,
            "cq": cq_t, "sq": sq_t, "ck": ck_t, "sk": sk_t,
            "pswap": pswap, "cmask": cmask,
        })
    return in_maps


def _run(nc, in_maps):
    if os.environ.get("BASS_SIM"):
        from concourse.bass_interp import CoreSim
        outs = []
        ncores = int(os.environ.get("BASS_SIM_CORES", "8"))
        for i, m in enumerate(in_maps[:ncores]):
            sim = CoreSim(nc, require_finite=False, require_nnan=False)
            for k, v in m.items():
                sim.tensor(k)[:] = v
            sim.simulate(check_with_hw=False)
            outs.append({"y": np.array(sim.tensor("y"))})
        while len(outs) < len(in_maps):
            outs.append({"y": np.zeros((S, D), np.float32)})
        return outs
    from concourse.bass_utils import run_bass_kernel_spmd
    res = run_bass_kernel_spmd(nc, in_maps, list(range(NCORES)))
    return res.results


def kernel(x, wq, bq, wk, bk, wv, bv, wo, bo):
    with_qk_bias = bool(np.any(np.asarray(bq)) or np.any(np.asarray(bk)))
    nc = _get_program(with_qk_bias)
    in_maps = _host_prep(x, wq, bq, wk, bk, wv, bv, wo, bo)
    results = _run(nc, in_maps)
    bv = np.asarray(bv, np.float32)
    bo = np.asarray(bo, np.float32)
    wo_f = np.asarray(wo, np.float32)
    corr = bv @ wo_f + bo  # [D]
    y = np.zeros((B, S, D), np.float32)
    for core in range(NCORES):
        b = core // HEADS_PER_CORE
        y[b] += results[core]["y"]
    y += corr[None, None, :]
    return y


# revision 13
# speedup vs baseline: 1.2553x; 1.0854x over previous
"""Multi-head attention (B=2, S=2048, D=1024, H=16, RoPE, causal) on 8 trn2 cores.

Sharding: core = b*4 + g  ->  batch b in {0,1}, head-group g (4 heads of 64 dims).
Each core computes q/k/v projections for its 4 heads, RoPE, causal attention,
and a partial output projection (its 256 rows of wo). Host sums the 4 partials
per batch and adds the analytic bias correction bv@wo + bo (softmax rows sum
to 1, so bv contributes a constant vector; bo is a constant vector).

Device layouts are feature-on-partition ("transposed"):
  xt [128, 8, 2048]   xt[p, ko, s] = x[b, s, ko*128 + p]            (bf16)
  qT/kT computed directly as [d', s]; RoPE pair-swap becomes a 32-block
  partition swap because wq/wk columns are host-permuted to [evens|odds]
  per head (valid: scores are invariant under a shared permutation of q,k).
  The swap itself is a PE matmul with a 128x128 XOR-32 permutation matrix.
  scoresT[j, i] = kT.T @ qT per head; two heads (one 128-part chunk) run as
  concurrent K=64 row-group matmuls into the two banks of one [128,1024]
  PSUM tile, so exp / mask / normalize handle both heads per op.
  Softmax skips max-subtraction (|score| <~ 8 here); exp on ACT with the
  1/sqrt(64) scale pre-folded into the q cos/sin tables. The denominator
  comes free from a ones-column appended to v in the AV matmul (out rows
  0..63 = v.T @ attnT, row 64 = column sums). Causality: blocks above the
  diagonal are skipped, diagonal blocks compute only columns >= 128*r and
  mask a single 128-wide strip.
  y[s, e] = outT.T @ wo accumulated over the 2 c-chunks, DMA'd PSUM->DRAM.
"""

import os

import numpy as np
import ml_dtypes

import concourse.bass as bass
import concourse.bacc as bacc
import concourse.tile as tile
from concourse import mybir

B = 2
S = 2048
D = 1024
H = 16
HD = 64
NCORES = 8
HEADS_PER_CORE = 4
DP = 256  # head dims per core
SEG = 512  # i-seg / s-seg size
NSEG = S // SEG  # 4
NST = S // 128  # 16 s-tiles / j-tiles
KO = D // 128  # 8 contraction tiles

F32 = mybir.dt.float32
BF16 = mybir.dt.bfloat16

_PROGRAMS = {}


def _build_program(with_qk_bias):
    # Pin the activation table to the one set containing Exp AND Ln (plus
    # copy/identity): the default greedy table placement thrashes between
    # exp_and_others and natural_log (17 ACT_TABLE_LOADs, 1.3us each).
    # Patched only for the duration of the build, then restored.
    import concourse.bacc as _bacc_mod
    orig_get_tables = _bacc_mod.get_activation_tables

    def _pinned_tables(arch):
        tabs = orig_get_tables(arch)
        if "natural_log_exp_and_others" not in tabs:
            return tabs
        # ids are positional (index into act_info.json) — keep every entry,
        # but empty the others so the chooser can only pick the pinned set
        return {k: (v if k == "natural_log_exp_and_others" else set())
                for k, v in tabs.items()}

    _bacc_mod.get_activation_tables = _pinned_tables
    try:
        return _build_program_inner(with_qk_bias)
    finally:
        _bacc_mod.get_activation_tables = orig_get_tables


def _build_program_inner(with_qk_bias):
    nc = bacc.Bacc("TRN2", target_bir_lowering=False, debug=False)

    xt_d = nc.dram_tensor("xt", [128, KO, S], BF16, kind="ExternalInput")
    wq_d = nc.dram_tensor("wqt", [128, KO, DP], BF16, kind="ExternalInput")
    wk_d = nc.dram_tensor("wkt", [128, KO, DP], BF16, kind="ExternalInput")
    wv_d = nc.dram_tensor("wvt", [128, KO, DP], BF16, kind="ExternalInput")
    wo_d = nc.dram_tensor("wot", [128, 2, D], BF16, kind="ExternalInput")
    bq_d = nc.dram_tensor("bqt", [128, 2], F32, kind="ExternalInput")
    bk_d = nc.dram_tensor("bkt", [128, 2], F32, kind="ExternalInput")
    cq_d = nc.dram_tensor("cq", [128, S], BF16, kind="ExternalInput")
    sq_d = nc.dram_tensor("sq", [128, S], BF16, kind="ExternalInput")
    ck_d = nc.dram_tensor("ck", [128, S], BF16, kind="ExternalInput")
    sk_d = nc.dram_tensor("sk", [128, S], BF16, kind="ExternalInput")
    pm_d = nc.dram_tensor("pswap", [128, 128], BF16, kind="ExternalInput")
    cm_d = nc.dram_tensor("cmask", [128, 128], BF16, kind="ExternalInput")
    y_d = nc.dram_tensor("y", [S, D], F32, kind="ExternalOutput")

    with tile.TileContext(nc) as tc:
        with (
            tc.tile_pool(name="const", bufs=1) as const,
            tc.tile_pool(name="persist", bufs=1) as persist,
            tc.tile_pool(name="work", bufs=4) as work,
            tc.tile_pool(name="psmm", bufs=2, space="PSUM") as psmm,
            tc.tile_pool(name="pssc", bufs=2, space="PSUM") as pssc,
            tc.tile_pool(name="psacc", bufs=2, space="PSUM") as psacc,
        ):
            # ---- constants ----
            xt = []
            for t in range(NSEG):
                xt_t = const.tile([128, KO, SEG], BF16, tag=f"xt{t}")
                nc.sync.dma_start(xt_t[:], xt_d[:, :, t * SEG:(t + 1) * SEG])
                xt.append(xt_t)
            wq = const.tile([128, KO, DP], BF16, tag="wq")
            nc.sync.dma_start(wq[:], wq_d[:])
            wk = const.tile([128, KO, DP], BF16, tag="wk")
            nc.sync.dma_start(wk[:], wk_d[:])
            wv = const.tile([128, KO, DP], BF16, tag="wv")
            nc.sync.dma_start(wv[:], wv_d[:])
            wo = const.tile([128, 2, D], BF16, tag="wo")
            nc.sync.dma_start(wo[:], wo_d[:])
            if with_qk_bias:
                bq = const.tile([128, 2], F32, tag="bq")
                nc.sync.dma_start(bq[:], bq_d[:])
                bk = const.tile([128, 2], F32, tag="bk")
                nc.sync.dma_start(bk[:], bk_d[:])
            tabs = {}
            for nm, dd in (("cq", cq_d), ("sq", sq_d), ("ck", ck_d), ("sk", sk_d)):
                tt = const.tile([128, S], BF16, tag=nm)
                nc.sync.dma_start(tt[:], dd[:])
                tabs[nm] = tt
            pm = const.tile([128, 128], BF16, tag="pm")
            nc.sync.dma_start(pm[:], pm_d[:])
            cm = const.tile([128, 128], BF16, tag="cm")
            nc.sync.dma_start(cm[:], cm_d[:])

            # ---- phase A: q/k projections + rope (per chunk c, seg t) ----
            qrot = {}
            krot = {}
            for c in range(2):
                for t in range(NSEG):
                    for which, w_sb, bofs, ctab, stab, store in (
                        ("q", wq, 0, tabs["cq"], tabs["sq"], qrot),
                        ("k", wk, 1, tabs["ck"], tabs["sk"], krot),
                    ):
                        pp = psmm.tile([128, SEG], F32, tag="mm",
                                       name=f"p{which}_{c}_{t}")
                        for ko in range(KO):
                            nc.tensor.matmul(
                                pp[:],
                                w_sb[:, ko, c * 128:(c + 1) * 128],
                                xt[t][:, ko, :],
                                start=(ko == 0),
                                stop=(ko == KO - 1),
                            )
                        qsb = work.tile([128, SEG], BF16, tag="qsb")
                        if with_qk_bias:
                            b_sb = bq if which == "q" else bk
                            nc.vector.tensor_scalar_add(
                                qsb[:], pp[:], b_sb[:, c:c + 1])
                        else:
                            nc.vector.tensor_copy(qsb[:], pp[:])
                        psw = psmm.tile([128, SEG], F32, tag="mm",
                                        name=f"psw{which}_{c}_{t}")
                        nc.tensor.matmul(psw[:], pm[:], qsb[:], start=True, stop=True)
                        t1 = work.tile([128, SEG], BF16, tag="t1")
                        nc.vector.tensor_tensor(
                            t1[:], qsb[:], ctab[:, t * SEG:(t + 1) * SEG],
                            mybir.AluOpType.mult)
                        t2 = work.tile([128, SEG], BF16, tag="t2")
                        nc.vector.tensor_tensor(
                            t2[:], psw[:], stab[:, t * SEG:(t + 1) * SEG],
                            mybir.AluOpType.mult)
                        rot = persist.tile([128, SEG], BF16, tag=f"{which}rot_{c}_{t}")
                        nc.vector.tensor_tensor(
                            rot[:], t1[:], t2[:], mybir.AluOpType.add)
                        store[(c, t)] = rot

            # ---- phase A2: v projection (natural layout + ones column) ----
            vt = []
            for st in range(NST):
                pv = psmm.tile([128, SEG], F32, tag="mm", name=f"pv_{st}")
                for ko in range(KO):
                    nc.tensor.matmul(
                        pv[:, :DP],
                        xt[st // NSEG][:, ko, (st % NSEG) * 128:(st % NSEG) * 128 + 128],
                        wv[:, ko, :],
                        start=(ko == 0),
                        stop=(ko == KO - 1),
                    )
                v_t = persist.tile([128, HEADS_PER_CORE, 66], BF16, tag=f"v_{st}")
                nc.vector.memset(v_t[:, :, 64:66], 1.0)
                nc.vector.tensor_copy(
                    v_t[:, :, 0:64],
                    pv[:, :DP].rearrange("p (h d) -> p h d", h=HEADS_PER_CORE))
                vt.append(v_t)

            # ---- phase B + C interleaved by i-seg t ----
            outt = {}
            for t in range(NSEG):
                for c in range(2):
                    pav = [psacc.tile([128, SEG], F32, tag="av",
                                      name=f"av_{c}_{t}_{par}")
                           for par in range(2)]
                    njt = 4 * t + 4
                    for jj in range(njt):
                        r = jj - 4 * t  # >= 0 on diagonal blocks
                        col0 = max(0, r) * 128  # first useful i-column
                        a = work.tile([128, 2, SEG], BF16, tag="attn")
                        ps = pssc.tile([128, 2, SEG], F32, tag="sc",
                                       name=f"sc_{c}_{t}_{jj}")
                        for par in range(2):
                            lo, hi = par * 64, par * 64 + 64
                            nc.tensor.matmul(
                                ps[:, par, col0:],
                                krot[(c, jj // 4)][lo:hi,
                                                   (jj % 4) * 128:(jj % 4) * 128 + 128],
                                qrot[(c, t)][lo:hi, col0:],
                                start=True, stop=True)
                        nc.scalar.activation(
                            a[:, :, col0:], ps[:, :, col0:],
                            mybir.ActivationFunctionType.Exp)
                        if r >= 0:
                            # mask the 128-wide diagonal strip (both parities)
                            nc.vector.tensor_tensor(
                                a[:, :, col0:col0 + 128], a[:, :, col0:col0 + 128],
                                cm[:, None, :].to_broadcast((128, 2, 128)),
                                mybir.AluOpType.mult)
                        for par in range(2):
                            nc.tensor.matmul(
                                pav[par][0:65, col0:],
                                vt[jj][:, 2 * c + par, 0:65],
                                a[:, par, col0:],
                                start=(jj == 0), stop=(jj == njt - 1))
                    ot = persist.tile([128, SEG], BF16, tag=f"outt_{c}_{t}")
                    outt[(c, t)] = ot
                    for par in range(2):
                        # copy out of PSUM right away so the accumulator bank
                        # frees for the next (c,t) j-loop; normalize off SBUF
                        u = work.tile([65, SEG], F32, tag="uav")
                        nc.any.tensor_copy(u[:], pav[par][0:65, :])
                        # 1/den = exp(-ln(den)) on ACT (reciprocal is an
                        # 8-cycle iterative op on DVE; exp+ln share one table)
                        lg = work.tile([1, SEG], F32, tag="lg")
                        nc.scalar.activation(
                            lg[:], u[64:65, :],
                            mybir.ActivationFunctionType.Ln)
                        rec = work.tile([1, SEG], F32, tag="rec")
                        nc.scalar.activation(
                            rec[:], lg[:],
                            mybir.ActivationFunctionType.Exp, scale=-1.0)
                        bc = work.tile([64, SEG], F32, tag="bc")
                        nc.gpsimd.partition_broadcast(bc[:], rec[:])
                        nc.vector.tensor_tensor(
                            ot[par * 64:par * 64 + 64, :],
                            u[0:64, :], bc[:], mybir.AluOpType.mult)
                # ---- output projection for the 4 s-tiles of this seg ----
                for sl in range(4):
                    st = 4 * t + sl
                    for es in range(2):
                        py = psmm.tile([128, SEG], F32, tag="mm",
                                       name=f"py_{st}_{es}")
                        for co in range(2):
                            nc.tensor.matmul(
                                py[:],
                                outt[(co, t)][:, sl * 128:sl * 128 + 128],
                                wo[:, co, es * SEG:(es + 1) * SEG],
                                start=(co == 0), stop=(co == 1))
                        ysb = work.tile([128, SEG], F32, tag="ysb")
                        nc.any.tensor_copy(ysb[:], py[:])
                        nc.sync.dma_start(
                            y_d[st * 128:(st + 1) * 128, es * SEG:(es + 1) * SEG],
                            ysb[:])

    nc.compile()
    return nc


def _get_program(with_qk_bias=False):
    if with_qk_bias not in _PROGRAMS:
        _PROGRAMS[with_qk_bias] = _build_program(with_qk_bias)
    return _PROGRAMS[with_qk_bias]


def _host_prep(x, wq, bq, wk, bk, wv, bv, wo, bo):
    """Build the 8 per-core input maps (all host-side numpy, cheap)."""
    bf = ml_dtypes.bfloat16
    x = np.asarray(x, np.float32)
    wq = np.asarray(wq, np.float32)
    wk = np.asarray(wk, np.float32)
    wv = np.asarray(wv, np.float32)
    wo = np.asarray(wo, np.float32)
    bq = np.asarray(bq, np.float32)
    bk = np.asarray(bk, np.float32)

    # rope tables, permuted-layout: partition p -> pair index m = p % 32,
    # first half of each 64-block (p%64<32) holds "evens", second "odds".
    m = np.arange(32, dtype=np.float64)
    inv_freq = 1.0 / (10000.0 ** (2.0 * m / HD))  # [32]
    pos = np.arange(S, dtype=np.float64)
    ang = pos[None, :] * inv_freq[:, None]  # [32, S]
    cos32 = np.cos(ang)
    sin32 = np.sin(ang)
    p = np.arange(128)
    cfull = cos32[p % 32, :]  # [128, S]
    sgn = np.where((p % 64) < 32, -1.0, 1.0)[:, None]
    sfull = sin32[p % 32, :] * sgn
    scale = 1.0 / np.sqrt(HD)
    cq_t = (cfull * scale).astype(bf)
    sq_t = (sfull * scale).astype(bf)
    ck_t = cfull.astype(bf)
    sk_t = sfull.astype(bf)

    pswap = np.zeros((128, 128), np.float32)
    pswap[np.arange(128), np.arange(128) ^ 32] = 1.0
    pswap = pswap.astype(bf)

    cmask = (p[:, None] <= np.arange(128)[None, :]).astype(bf)  # keep p <= i'

    in_maps = []
    for core in range(NCORES):
        b, g = divmod(core, HEADS_PER_CORE)
        # permuted columns for q/k: per head [evens, odds]
        colmap = np.concatenate([
            (4 * g + hl) * HD + np.concatenate([np.arange(0, HD, 2),
                                                np.arange(1, HD, 2)])
            for hl in range(HEADS_PER_CORE)
        ])  # [256] global col indices
        vcols = np.arange(g * DP, (g + 1) * DP)

        xt = np.ascontiguousarray(
            x[b].T.reshape(KO, 128, S).transpose(1, 0, 2)).astype(bf)
        wq_t = np.ascontiguousarray(
            wq[:, colmap].reshape(KO, 128, DP).transpose(1, 0, 2)).astype(bf)
        wk_t = np.ascontiguousarray(
            wk[:, colmap].reshape(KO, 128, DP).transpose(1, 0, 2)).astype(bf)
        wv_t = np.ascontiguousarray(
            wv[:, vcols].reshape(KO, 128, DP).transpose(1, 0, 2)).astype(bf)
        wo_t = np.ascontiguousarray(
            wo[vcols, :].reshape(2, 128, D).transpose(1, 0, 2)).astype(bf)
        bq_t = np.ascontiguousarray(bq[colmap].reshape(2, 128).T).astype(np.float32)
        bk_t = np.ascontiguousarray(bk[colmap].reshape(2, 128).T).astype(np.float32)

        in_maps.append({
            "xt": xt, "wqt": wq_t, "wkt": wk_t, "wvt": wv_t, "wot": wo_t,
            "bqt": bq_t, "bkt": bk_t,
            "cq": cq_t, "sq": sq_t, "ck": ck_t, "sk": sk_t,
            "pswap": pswap, "cmask": cmask,
        })
    return in_maps


def _run(nc, in_maps):
    if os.environ.get("BASS_SIM"):
        from concourse.bass_interp import CoreSim
        outs = []
        ncores = int(os.environ.get("BASS_SIM_CORES", "8"))
        for i, m in enumerate(in_maps[:ncores]):
            sim = CoreSim(nc, require_finite=False, require_nnan=False)
            for k, v in m.items():
                sim.tensor(k)[:] = v
            sim.simulate(check_with_hw=False)
            outs.append({"y": np.array(sim.tensor("y"))})
        while len(outs) < len(in_maps):
            outs.append({"y": np.zeros((S, D), np.float32)})
        return outs
    from concourse.bass_utils import run_bass_kernel_spmd
    res = run_bass_kernel_spmd(nc, in_maps, list(range(NCORES)))
    return res.results


def kernel(x, wq, bq, wk, bk, wv, bv, wo, bo):
    with_qk_bias = bool(np.any(np.asarray(bq)) or np.any(np.asarray(bk)))
    nc = _get_program(with_qk_bias)
    in_maps = _host_prep(x, wq, bq, wk, bk, wv, bv, wo, bo)
    results = _run(nc, in_maps)
    bv = np.asarray(bv, np.float32)
    bo = np.asarray(bo, np.float32)
    wo_f = np.asarray(wo, np.float32)
    corr = bv @ wo_f + bo  # [D]
    y = np.zeros((B, S, D), np.float32)
    for core in range(NCORES):
        b = core // HEADS_PER_CORE
        y[b] += results[core]["y"]
    y += corr[None, None, :]
    return y
